# revision 16
# baseline (speedup 1.0000x reference)
"""Masked multi-head attention on 8 trn2 NeuronCores (Bass/Tile).

Problem: B=1, N=4096, C=256, H=8 heads (Dh=32), dense 0/1 mask adj
shared across heads.  reference:
    qkv = x @ w_qkv.T ; q,k,v per head
    attn = softmax(where(adj>0, q@k.T*scale, -9e15))
    out  = (attn @ v) @ w_proj.T + b_proj

Sharding: sequence-parallel over query rows.  Core i handles query rows
[512*i, 512*(i+1)) for ALL heads; k/v are recomputed on every core from
the (small) full x.  The big tensor -- the 64MB mask -- is split 8 ways
with no replication and there are no collectives.

Per-core pipeline (bf16 matmuls, f32 PSUM):
  scores computed TRANSPOSED: sT[kpos, qrow] = k @ q^T, two heads per
  k-tile row-packed into the PE array (K=32 tile_position groups, one
  PSUM bank each -- concurrent row-grouped matmuls must NOT share a
  bank).  exp on ScalarE straight from PSUM (no max-subtraction needed:
  |s| <~ 7).  Mask applied multiplicatively after exp in ONE VectorE op
  per tile (mask duplicated for the two head-halves).  p @ v uses v
  augmented with a ones column (M=33) so the softmax denominator
  accumulates for free in the same matmul; heads col-packed 2-per-bank.
  Denominator reciprocals via reciprocal_approx_fast, broadcast across
  32 partitions by a K=1 PE matmul, normalization fused with the bf16
  downcast, final projection = plain K=128 accumulation against a
  host-side zero-padded reordering of w_proj.
"""

import sys

for _p in ("/opt/trn_rl_repo", "/root/.axon_site/_ro/trn_rl_repo"):
    if _p not in sys.path:
        sys.path.insert(0, _p)

import numpy as np
import ml_dtypes

BF16NP = ml_dtypes.bfloat16

N = 4096
C = 256
H = 8
DH = 32
NCORES = 8
NQ = N // NCORES  # 512 query rows per core
KT = N // 128  # 32 key tiles

_CACHE = {}


def build_kernel():
    import concourse.bacc as bacc
    import concourse.tile as tile
    from concourse import mybir

    F32 = mybir.dt.float32
    BF = mybir.dt.bfloat16
    EXP = mybir.ActivationFunctionType.Exp
    IDENT = mybir.ActivationFunctionType.Identity

    nc = bacc.Bacc("TRN2", target_bir_lowering=False, debug=False, num_devices=NCORES)

    xT_d = nc.dram_tensor("xT", [C, N], BF, kind="ExternalInput")
    xqT_d = nc.dram_tensor("xqT", [C, NQ], BF, kind="ExternalInput")
    wqkv_d = nc.dram_tensor("wqkv", [C, 3 * C], BF, kind="ExternalInput")
    wproj2_d = nc.dram_tensor("wproj2", [4 * 128, C], BF, kind="ExternalInput")
    bias2_d = nc.dram_tensor("bias2", [128, 2], F32, kind="ExternalInput")
    maskT_d = nc.dram_tensor("maskT", [N, 2 * NQ], BF, kind="ExternalInput")
    out_d = nc.dram_tensor("out", [C, NQ], F32, kind="ExternalOutput")

    with (
        tile.TileContext(nc) as tc,
        tc.tile_pool(name="consts", bufs=1) as consts,
        tc.tile_pool(name="ps", bufs=3, space="PSUM") as ps_pool,
        tc.tile_pool(name="pvps", bufs=2, space="PSUM") as pv_pool,
        tc.tile_pool(name="ptile", bufs=4) as p_pool,
    ):
        # ---------------- PE warmup ----------------
        # ~10us of junk matmuls run while input DMAs are in flight, so the
        # HAM clock gate is already at 8/8 when the real matmuls start.
        junk = consts.tile([128, 512], BF)
        nc.vector.memset(junk, 0.0)
        warm_ps = ps_pool.tile([128, 1024], F32, name="warm_ps", tag="s")
        for i in range(24):
            nc.tensor.matmul(
                out=warm_ps[:, 0:512],
                lhsT=junk[:, 0:128],
                rhs=junk[:, 0:512],
                start=True,
                stop=True,
            )

        # ---------------- input DMAs ----------------
        w_sb = [
            consts.tile([128, 3 * C], BF, name=f"w_sb{c}", tag=f"w{c}")
            for c in range(2)
        ]
        for c in range(2):
            nc.sync.dma_start(out=w_sb[c], in_=wqkv_d[128 * c : 128 * (c + 1), :])
        wp_sb = [
            consts.tile([128, C], BF, name=f"wp_sb{g2}", tag=f"wp{g2}")
            for g2 in range(4)
        ]
        for g2 in range(4):
            nc.sync.dma_start(
                out=wp_sb[g2], in_=wproj2_d[128 * g2 : 128 * (g2 + 1), :]
            )
        bias_sb = consts.tile([128, 2], F32)
        nc.sync.dma_start(out=bias_sb, in_=bias2_d[:])
        xq_sb = [
            consts.tile([128, NQ], BF, name=f"xq_sb{c}", tag=f"xq{c}") for c in range(2)
        ]
        for c in range(2):
            nc.sync.dma_start(out=xq_sb[c], in_=xqT_d[128 * c : 128 * (c + 1), :])
        xT_sb = [
            consts.tile([128, N], BF, name=f"xT_sb{c}", tag=f"xT{c}") for c in range(2)
        ]
        for c in range(2):
            for ch in range(4):
                cs = slice(ch * (N // 4), (ch + 1) * (N // 4))
                nc.sync.dma_start(
                    out=xT_sb[c][:, cs], in_=xT_d[128 * c : 128 * (c + 1), cs]
                )
        # mask (already duplicated for the two head-halves):
        # [4096, 2*NQ] -> sbuf [128, KT, 2*NQ]  (row m*128+p -> [p, m, :])
        mask_sb = consts.tile([128, KT, 2 * NQ], BF)
        maskT_r = maskT_d.rearrange("(m p) q -> p m q", p=128)
        n_chunks = min(8, KT)
        step = KT // n_chunks
        for ch in range(n_chunks):
            nc.sync.dma_start(
                out=mask_sb[:, step * ch : step * (ch + 1), :],
                in_=maskT_r[:, step * ch : step * (ch + 1), :],
            )

        ones_f = consts.tile([128, 32], F32)
        nc.vector.memset(ones_f, 1.0)

        # ---------------- phase 1: qT, kT, v ----------------
        qT_sb = [
            consts.tile([128, NQ], BF, name=f"qT_sb{g}", tag=f"qT{g}") for g in range(2)
        ]
        for g in range(2):
            q_ps = ps_pool.tile([128, 1024], F32, name="q_ps", tag="s")
            for c in range(2):
                nc.tensor.matmul(
                    out=q_ps[:, 0:NQ],
                    lhsT=w_sb[c][:, 128 * g : 128 * (g + 1)],
                    rhs=xq_sb[c],
                    start=(c == 0),
                    stop=(c == 1),
                )
            nc.vector.tensor_copy(out=qT_sb[g], in_=q_ps[:, 0:NQ])

        kT_sb = [
            consts.tile([128, N], BF, name=f"kT_sb{g}", tag=f"kT{g}") for g in range(2)
        ]

        def emit_kT(g, n):
            k_ps = ps_pool.tile([128, 1024], F32, name="k_ps", tag="s")
            for c in range(2):
                nc.tensor.matmul(
                    out=k_ps[:, 0:512],
                    lhsT=w_sb[c][:, 256 + 128 * g : 256 + 128 * (g + 1)],
                    rhs=xT_sb[c][:, 512 * n : 512 * (n + 1)],
                    start=(c == 0),
                    stop=(c == 1),
                )
            nc.vector.tensor_copy(
                out=kT_sb[g][:, 512 * n : 512 * (n + 1)], in_=k_ps[:, 0:512]
            )

        # v tiles interleaved per head with a ones column: 34-wide blocks
        # [v_h (32) | 1 | pad], so lhsT [128, 33] per head fuses the softmax
        # denominator into the pv matmul as output row 32.
        v_sb = [
            consts.tile([128, 34 * H], BF, name=f"v_sb_{m}", tag=f"v{m}")
            for m in range(KT)
        ]

        def emit_v(m):
            v_ps = ps_pool.tile([128, 1024], F32, name="v_ps", tag="s")
            for c in range(2):
                nc.tensor.matmul(
                    out=v_ps[:, 0:C],
                    lhsT=xT_sb[c][:, 128 * m : 128 * (m + 1)],
                    rhs=w_sb[c][:, 512:768],
                    start=(c == 0),
                    stop=(c == 1),
                )
            vt3 = v_sb[m].rearrange("p (h w) -> p h w", h=H)
            nc.vector.memset(vt3[:, :, 32:34], 1.0)
            nc.vector.tensor_copy(
                out=vt3[:, :, 0:32],
                in_=v_ps[:, 0:C].rearrange("p (h w) -> p h w", h=H),
            )

        for n in range(N // 512):
            emit_kT(0, n)

        # ---------------- phase 2: attention ----------------
        # One pv bank per head pair, allocated per-pair so only 2 banks are
        # live at a time (double-buffered across pair boundaries): head A
        # rows 0:32 + denom row 32, head B rows 64:96 + denom row 96
        # (M=33 col groups at 0 and 64).
        rec_sb = consts.tile([128, 4 * 512], F32)
        bc_cat = consts.tile([128, 4 * 512], F32)
        nc.vector.memset(bc_cat, 0.0)
        o_cat = [
            consts.tile([128, NQ], BF, name=f"o_cat{g2}", tag=f"oc{g2}")
            for g2 in range(4)
        ]
        o_raw = [
            consts.tile([128, NQ], F32, name=f"o_raw{g2}", tag=f"or{g2}")
            for g2 in range(4)
        ]

        def emit_norm(j):
            # normalization for pair j: reciprocal of denom rows (full-tile:
            # the approx op misbehaves on 1-row slices on HW), PE broadcast
            # of rows 32/96 across each head's 32 partitions, multiply +
            # bf16 downcast.
            nc.vector.reciprocal_approx_fast(
                out=rec_sb[:, 512 * j : 512 * j + NQ],
                in_=o_raw[j],
            )
            for prow, orow in ((32, 0), (96, 64)):
                bc_ps = ps_pool.tile([128, 1024], F32, name="bc_ps", tag="s")
                nc.tensor.matmul(
                    out=bc_ps[orow : orow + 32, 0:NQ],
                    lhsT=ones_f[prow : prow + 1, :],
                    rhs=rec_sb[prow : prow + 1, 512 * j : 512 * j + NQ],
                    start=True,
                    stop=True,
                    tile_position=(prow, orow),
                )
                nc.vector.tensor_copy(
                    out=bc_cat[orow : orow + 32, 512 * j : 512 * j + NQ],
                    in_=bc_ps[orow : orow + 32, 0:NQ],
                )
            nc.vector.tensor_mul(
                out=o_cat[j],
                in0=o_raw[j],
                in1=bc_cat[:, 512 * j : 512 * j + NQ],
            )

        for g2 in range(4):  # head pairs
            hA, hB = 2 * g2, 2 * g2 + 1
            gA, pA = hA // 4, 32 * (hA % 4)
            gB, pB = hB // 4, 32 * (hB % 4)
            pv_t = pv_pool.tile([128, 512], F32, name="pv_t", tag="pv")
            # 1.0 keeps reciprocal_approx_fast well-defined on unwritten rows
            nc.vector.memset(pv_t, 1.0)
            for m in range(KT):
                # lazy projection: v and kT[1] are computed during the first
                # pairs' loops so the PE never sits in a long phase-1 block
                if g2 == 1 and m % 4 == 0:
                    emit_kT(1, m // 4)
                if g2 == 0:
                    emit_v(m)
                # previous pair's normalization, off the critical path
                if m == 2 and g2 > 0:
                    emit_norm(g2 - 1)
                s_ps = ps_pool.tile([128, 1024], F32, name="s_ps", tag="s")
                nc.tensor.matmul(
                    out=s_ps[:, 0:NQ],
                    lhsT=kT_sb[gA][pA : pA + 32, 128 * m : 128 * (m + 1)],
                    rhs=qT_sb[gA][pA : pA + 32, :],
                    start=True,
                    stop=True,
                    tile_position=(pA, 0),
                )
                nc.tensor.matmul(
                    out=s_ps[:, 512 : 512 + NQ],
                    lhsT=kT_sb[gB][pB : pB + 32, 128 * m : 128 * (m + 1)],
                    rhs=qT_sb[gB][pB : pB + 32, :],
                    start=True,
                    stop=True,
                    tile_position=(pB, 0),
                )
                p_sb = p_pool.tile([128, 2 * NQ], BF, name="p_sb", tag="p")
                if NQ == 512:
                    nc.scalar.activation(out=p_sb, in_=s_ps, func=EXP)
                else:
                    nc.scalar.activation(
                        out=p_sb[:, 0:NQ], in_=s_ps[:, 0:NQ], func=EXP
                    )
                    nc.scalar.activation(
                        out=p_sb[:, NQ : 2 * NQ], in_=s_ps[:, 512 : 512 + NQ], func=EXP
                    )
                nc.vector.tensor_mul(out=p_sb, in0=p_sb, in1=mask_sb[:, m, :])
                first, last = (m == 0), (m == KT - 1)
                nc.tensor.matmul(
                    out=pv_t[0:33, 0:NQ],
                    lhsT=v_sb[m][:, 34 * hA : 34 * hA + 33],
                    rhs=p_sb[:, 0:NQ],
                    start=first,
                    stop=last,
                    tile_position=(0, 0),
                    skip_group_check=True,
                )
                nc.tensor.matmul(
                    out=pv_t[64:97, 0:NQ],
                    lhsT=v_sb[m][:, 34 * hB : 34 * hB + 33],
                    rhs=p_sb[:, NQ : 2 * NQ],
                    start=first,
                    stop=last,
                    tile_position=(0, 64),
                    skip_group_check=True,
                )

            # pair epilogue: just copy the raw pv bank to SBUF so the bank
            # frees immediately; all normalization happens after the loop,
            # off the critical path.
            nc.vector.tensor_copy(
                out=o_raw[g2], in_=pv_t[:, 0:NQ]
            )

        emit_norm(3)

        # final projection: full K=128 accumulating matmuls against the
        # host-side zero-padded reordering of w_proj^T.
        for t in range(2):
            f_ps = ps_pool.tile([128, 1024], F32, name="f_ps", tag="s")
            for g2 in range(4):
                nc.tensor.matmul(
                    out=f_ps[:, 0:NQ],
                    lhsT=wp_sb[g2][:, 128 * t : 128 * (t + 1)],
                    rhs=o_cat[g2],
                    start=(g2 == 0),
                    stop=(g2 == 3),
                )
            fin = consts.tile([128, NQ], F32, name=f"fin{t}", tag=f"fin{t}")
            nc.scalar.activation(
                out=fin,
                in_=f_ps[:, 0:NQ],
                func=IDENT,
                bias=bias_sb[:, t : t + 1],
                scale=1.0,
            )
            nc.sync.dma_start(out=out_d[128 * t : 128 * (t + 1), :], in_=fin)

    nc.compile()
    return nc


def _get_nc():
    if "nc" not in _CACHE:
        _CACHE["nc"] = build_kernel()
    return _CACHE["nc"]


def _prep_inputs(x, adj, w_qkv, w_proj, b_proj):
    x = np.asarray(x, dtype=np.float32).reshape(N, C)
    adj = np.asarray(adj).reshape(N, N)
    w_qkv = np.asarray(w_qkv, dtype=np.float32)
    w_proj = np.asarray(w_proj, dtype=np.float32)
    b_proj = np.asarray(b_proj, dtype=np.float32)

    scale = float(DH) ** -0.5
    wqkvT = w_qkv.T.copy()
    wqkvT[:, 0:C] *= scale  # fold attention scale into q projection
    wqkvT = np.ascontiguousarray(wqkvT, dtype=BF16NP)
    wprojT = w_proj.T.astype(np.float32)  # [C (contraction), C (out)]
    # zero-padded reorder: block g2 rows 0:32 = head 2*g2, rows 64:96 =
    # head 2*g2+1, rest zero (matches pv bank partition layout)
    wproj2 = np.zeros((4 * 128, C), dtype=np.float32)
    for g2 in range(4):
        wproj2[128 * g2 + 0 : 128 * g2 + 32] = wprojT[64 * g2 : 64 * g2 + 32]
        wproj2[128 * g2 + 64 : 128 * g2 + 96] = wprojT[64 * g2 + 32 : 64 * g2 + 64]
    wproj2 = np.ascontiguousarray(wproj2, dtype=BF16NP)
    bias2 = np.ascontiguousarray(b_proj.reshape(2, 128).T, dtype=np.float32)
    xT = np.ascontiguousarray(x.T, dtype=BF16NP)
    adjT = (adj > 0).astype(BF16NP).T  # [kpos, qrow] 0/1

    in_maps = []
    for i in range(NCORES):
        sl = slice(NQ * i, NQ * (i + 1))
        m1 = np.ascontiguousarray(adjT[:, sl])
        in_maps.append(
            {
                "xT": xT,
                "xqT": np.ascontiguousarray(xT[:, sl]),
                "wqkv": wqkvT,
                "wproj2": wproj2,
                "bias2": bias2,
                "maskT": np.ascontiguousarray(np.concatenate([m1, m1], axis=1)),
            }
        )
    return in_maps


def run_on_hw(inputs, trace=False):
    from concourse.bass_utils import run_bass_kernel_spmd

    if trace:
        import axon_profile_shim  # noqa: F401

    nc = _get_nc()
    in_maps = _prep_inputs(**inputs)
    res = run_bass_kernel_spmd(
        nc, in_maps, core_ids=list(range(NCORES)), trace=trace
    )
    out = np.empty((1, N, C), dtype=np.float32)
    for i in range(NCORES):
        out[0, NQ * i : NQ * (i + 1), :] = res.results[i]["out"].T
    return out, res


def kernel(x, adj, w_qkv, w_proj, b_proj):
    out, _ = run_on_hw(
        {"x": x, "adj": adj, "w_qkv": w_qkv, "w_proj": w_proj, "b_proj": b_proj}
    )
    return out
